# revision 18
# baseline (speedup 1.0000x reference)
"""CrossAttention Trainium2 Bass kernel.

Problem: x[4,256,64,64], a[4,256,32,32], Wq[512,256], Wkv[1024,256],
Wout[256,512], bout[256] -> y[4,256,64,64]  (8 heads, dim_head 64).

Sharding: 8 cores = (batch b in 0..3) x (query-half in 0..1). Each core
computes all 8 heads for a [256, 2048] slice of x (2048 query positions)
against the full [256, 1024] kv field of its batch, and produces the
complete [256, 2048] output slice (no cross-core reduction needed).

Device-side math per core (matmuls in float32r):
  Q  = (0.125*Wq)^T.T @ X      [512, 2048]   (scale folded into Wq on host)
  K  = Wk^T.T @ A              [512, 1024]
  VT = A-chunks.T @ Wv^T       [1024, 520]   (j on partitions, per-head ones
                                              column appended -> Z row)
  simT[j,i] per head pair: two row-tiled K=64 matmuls (tile_position (0,0)
      and (64,0)) write head A to psum cols 0:512 and head B to 512:1024.
  expT: 6 of 8 j-chunks exponentiate on the ACT engine (true exp, bf16 out);
      2 chunks ride the DVE as a Schraudolph add in the bf16 bit domain
      (exp(x) ~= bitcast_bf16(int16(x*A + B)); x*A folded into Wq).
  AV: av[65, i] = vt_aug.T @ expT accumulated over j-chunks; row 64 = Z.
  otn = av * (1/Z) via DVE reciprocal + gpsimd broadcast + DVE mult
  Y  = Wout^T.T @ otn accumulated over head pairs in PSUM, + bout, at end.

Schedule: the PE's total work (~311k cycles ~= 130us) is the hard floor
(sim matmuls run at half array utilization because d=64; fp8 DoubleRow
cannot fix that, and fp8 AV fails the 2e-2 accuracy budget - measured).
So the schedule keeps the PE saturated end to end:
  - inputs stream on 5 DGE rings, ordered by first use; ACT's exp table
    preloads behind the DMA head.
  - pair-0 K/Q project first; stage 0's empty AV slots run the VT
    projections (own PSUM rotation in psAv, no sim-psum contention);
    pairs 1-3 K/Q spread one chunk per stage over stages 1-9.
  - per stage 6 exps on ACT (6.2us) + 2 on DVE (2.4us + 2.6us normalize
    work) both sit under the PE's 6.83us/stage.
"""

import numpy as np

HEADS = 8
DH = 64
HID = 512
CQ = 256
CKV = 256
B = 4
HW = 4096
IC = 2048  # query positions per core
NJ = 1024  # kv positions
P = 128

# Schraudolph exp in the bf16 bit domain:
#   exp(x) ~= bitcast_bf16(int16(x * A + B)), max rel err ~3.4%
SCH_A = 184.6650390625      # 2^7 / ln 2
SCH_B = 16250.66            # 127 * 2^7 - c_adj (robust to trunc/round)

import os as _os
# Which of the 8 exp chunks per stage run on the DVE as a Schraudolph add.
# Default NONE: on real HW the DVE tensor_scalar reading PSUM runs far
# below the cost model (a 2-chunk split measured +37us vs all-ACT), so all
# exps stay on the ACT engine.
DVE_JC = tuple(int(c) for c in _os.environ.get("KRN_DVE_JC", ""))
COPY_ENG = _os.environ.get("KRN_COPY", "dve")     # proj psum-drain engine
STAGGER = int(_os.environ.get("KRN_STAGGER", "0"))  # pairs 1-3 K/Q in-stage
# PSUM3 (3 sim-psum bufs / 1-slot normalize / 2 av bufs) helps in CoreSim
# but regresses on HW: the 1-slot normalize puts the Pool broadcast onto
# the per-stage av-buffer critical chain. Keep the 2-slot normalize.
PSUM3 = int(_os.environ.get("KRN_PSUM3", "0"))

_RUNNER = None


def _build_nc(reps=1):
    import concourse.bass as bass
    import concourse.mybir as mybir
    from concourse import tile, bacc
    from concourse.bass_interp import get_hw_module

    f32 = mybir.dt.float32
    f32r = mybir.dt.float32r
    bf16 = mybir.dt.bfloat16
    i16 = mybir.dt.int16
    AF = mybir.ActivationFunctionType
    ALU = mybir.AluOpType

    nc = bacc.Bacc("TRN2", target_bir_lowering=False, debug=False, num_devices=8)

    x_d = nc.dram_tensor("x", [CQ, IC], f32r, kind="ExternalInput")
    a_d = nc.dram_tensor("a", [CKV, NJ], f32r, kind="ExternalInput")
    wq_d = nc.dram_tensor("wq", [CQ, HID], f32r, kind="ExternalInput")
    wk_d = nc.dram_tensor("wk", [CKV, HID], f32r, kind="ExternalInput")
    wv_d = nc.dram_tensor("wv", [CKV, HID], f32r, kind="ExternalInput")
    wo_d = nc.dram_tensor("wo", [HID, CQ], f32r, kind="ExternalInput")
    bo_d = nc.dram_tensor("bo", [CQ, 1], f32, kind="ExternalInput")
    ones_d = nc.dram_tensor("ones", [P, HEADS], f32r, kind="ExternalInput")
    y_d = nc.dram_tensor("y", [CQ, IC], f32, kind="ExternalOutput")

    def body(nc, tc, pools):
        wpool, qpool, kpool, vpool, epool, opool, spool, psSim, psAv = pools

        # ---- ACT exp-table preload: a dummy 1-elem exp issued first makes
        # the 1.3us table load ride the DMA head instead of the first real
        # exp on the critical path.
        sc = spool.tile([1, 2], f32, tag="sc", name="sc")
        nc.vector.memset(sc[:, 0:1], 0.0)
        nc.scalar.activation(sc[0:1, 1:2], sc[0:1, 0:1], AF.Exp)

        # ---- input DMAs on the 3 DGE rings (SP, ACT, GPSIMD), ordered by
        # first use: a (K/VT input) on sync+gpsimd, wk+wq on scalar (K0/Q0
        # weights), then the x halves (Q proj input, gates the pipeline
        # head) round-robin; wv/wo/bo ride behind (not read until VT / the
        # output projection).
        a_sb = [wpool.tile([P, NJ], f32r, tag=f"a{kc}", name=f"a{kc}")
                for kc in range(2)]
        for kc in range(2):
            for half in range(2):
                ring = nc.sync if (kc + half) % 2 == 0 else nc.gpsimd
                ring.dma_start(
                    a_sb[kc][:, half * 512:(half + 1) * 512],
                    a_d[kc * P:(kc + 1) * P, half * 512:(half + 1) * 512])
        wk_sb = [wpool.tile([P, HID], f32r, tag=f"wk{kc}", name=f"wk{kc}")
                 for kc in range(2)]
        wq_sb = [wpool.tile([P, HID], f32r, tag=f"wq{kc}", name=f"wq{kc}")
                 for kc in range(2)]
        for kc in range(2):
            nc.scalar.dma_start(wk_sb[kc][:], wk_d[kc * P:(kc + 1) * P, :])
        for kc in range(2):
            nc.scalar.dma_start(wq_sb[kc][:], wq_d[kc * P:(kc + 1) * P, :])
        x_sb = [wpool.tile([P, IC], f32r, tag=f"x{kc}", name=f"x{kc}")
                for kc in range(2)]
        xring = [nc.sync, nc.gpsimd, nc.scalar, nc.sync]
        for half in range(2):
            for kc in range(2):
                xring[half * 2 + kc].dma_start(
                    x_sb[kc][:, half * 1024:(half + 1) * 1024],
                    x_d[kc * P:(kc + 1) * P, half * 1024:(half + 1) * 1024])
        wv_sb = []
        for kc in range(2):
            t = wpool.tile([P, HID], f32r, tag=f"wv{kc}", name=f"wv{kc}")
            nc.gpsimd.dma_start(t[:], wv_d[kc * P:(kc + 1) * P, :])
            wv_sb.append(t)
        wo_sb = []
        wo_ring = [nc.scalar, nc.sync, nc.scalar, nc.sync]
        for pc in range(4):
            t = wpool.tile([P, CQ], f32r, tag=f"wo{pc}", name=f"wo{pc}")
            wo_ring[pc].dma_start(t[:], wo_d[pc * P:(pc + 1) * P, :])
            wo_sb.append(t)
        bo_sb = []
        for mc in range(2):
            t = wpool.tile([P, 1], f32, tag=f"bo{mc}", name=f"bo{mc}")
            nc.gpsimd.dma_start(t[:], bo_d[mc * P:(mc + 1) * P, :])
            bo_sb.append(t)

        # ---- projections ----
        # matmul(out, lhsT, rhs): out = lhsT.T @ rhs.
        q_sb = [qpool.tile([P, IC], f32r, tag=f"q{mc}", name=f"q{mc}")
                for mc in range(4)]
        k_sb = [kpool.tile([P, NJ], f32r, tag=f"k{mc}", name=f"k{mc}")
                for mc in range(4)]
        vt_sb = []

        def emit_proj_k(mc, copy_on_act=(COPY_ENG == "act")):
            ps = psSim.tile([P, 1024], f32, tag="sim", name="psk")
            for half in range(2):
                for kc in range(2):
                    nc.tensor.matmul(
                        ps[:, half * 512:(half + 1) * 512],
                        wk_sb[kc][:, mc * P:(mc + 1) * P],
                        a_sb[kc][:, half * 512:(half + 1) * 512],
                        start=(kc == 0), stop=(kc == 1),
                    )
            # psum-drain copies ride the ACT engine (it has headroom; the
            # DVE carries the exps + normalize and is the busier engine)
            if copy_on_act:
                nc.scalar.activation(k_sb[mc][:], ps[:], AF.Copy)
            else:
                nc.vector.tensor_copy(k_sb[mc][:], ps[:])

        def emit_proj_q(mc, n, copy_on_act=(COPY_ENG == "act")):
            ps = psSim.tile([P, 1024], f32, tag="sim", name="psq")
            for half in range(2):
                for kc in range(2):
                    nc.tensor.matmul(
                        ps[:, half * 512:(half + 1) * 512],
                        wq_sb[kc][:, mc * P:(mc + 1) * P],
                        x_sb[kc][:, n * 1024 + half * 512:
                                  n * 1024 + (half + 1) * 512],
                        start=(kc == 0), stop=(kc == 1),
                    )
            dst = q_sb[mc][:, n * 1024:(n + 1) * 1024]
            if copy_on_act:
                nc.scalar.activation(dst, ps[:], AF.Copy)
            else:
                nc.vector.tensor_copy(dst, ps[:])

        # VT[j, hd] = sum_c a[c, j] wv[c, hd] : [1024, 512], with per-head
        # ones column appended -> vt tiles [128, 520]. Emitted inside stage
        # 0's (empty) AV slots, on the psAv rotation so the sims' psum
        # buffers stay out of the way.
        for jc in range(8):
            vt_sb.append(vpool.tile([P, HEADS * (DH + 1)], bf16,
                                    tag=f"vt{jc}", name=f"vt{jc}"))

        def emit_vt2(j0):
            for jc in (j0, j0 + 1):
                vt = vt_sb[jc]
                ones_dst = vt[:].rearrange(
                    "p (h d) -> p h d", h=HEADS, d=DH + 1)[:, :, DH:DH + 1]
                nc.gpsimd.dma_start(ones_dst, ones_d[:].unsqueeze(-1))
                ps = psAv.tile([P, 512], f32, tag="av", name=f"vtp{jc}")
                for kc in range(2):
                    nc.tensor.matmul(
                        ps[:],
                        a_sb[kc][:, jc * P:(jc + 1) * P],
                        wv_sb[kc][:],
                        start=(kc == 0), stop=(kc == 1),
                    )
                dst = vt[:].rearrange(
                    "p (h d) -> p h d", h=HEADS, d=DH + 1)[:, :, 0:DH]
                src = ps[:].rearrange("p (h d) -> p h d", h=HEADS, d=DH)
                nc.vector.tensor_copy(dst, src)

        # pairs 1-3 K/Q spread one psSim-sized chunk per stage, stages 1-9
        # (pair p is first read at stage 4p, so every chunk lands early).
        stagger = [(emit_proj_k, (1,)), (emit_proj_q, (1, 0)),
                   (emit_proj_q, (1, 1)),
                   (emit_proj_k, (2,)), (emit_proj_q, (2, 0)),
                   (emit_proj_q, (2, 1)),
                   (emit_proj_k, (3,)), (emit_proj_q, (3, 0)),
                   (emit_proj_q, (3, 1))]

        # ---- attention stages ----
        otn_sb = [opool.tile([P, IC], f32r, tag=f"otn{p}", name=f"otn{p}")
                  for p in range(4)]

        def emit_sim_exp(s, pair, icq, jc):
            """Row-tiled sim pair (heads 2*pair, 2*pair+1) + exp."""
            ps = psSim.tile([P, 1024], f32, tag="sim", name="sim")
            nc.tensor.matmul(
                ps[:, 0:512],
                k_sb[pair][0:DH, jc * P:(jc + 1) * P],
                q_sb[pair][0:DH, icq * 512:(icq + 1) * 512],
                start=True, stop=True, tile_position=(0, 0),
            )
            nc.tensor.matmul(
                ps[:, 512:1024],
                k_sb[pair][DH:2 * DH, jc * P:(jc + 1) * P],
                q_sb[pair][DH:2 * DH, icq * 512:(icq + 1) * 512],
                start=True, stop=True, tile_position=(64, 0),
            )
            # expt double-buffered by stage parity: stage s+1's exps must
            # not overwrite the tiles stage s's AV matmuls still read.
            et = epool.tile([P, 1024], bf16, tag=f"e{jc}_{s % 2}",
                            name=f"e{jc}")
            if jc in DVE_JC:
                nc.vector.tensor_scalar(
                    et[:].bitcast(i16), ps[:], SCH_B, None, ALU.add)
            else:
                nc.scalar.activation(et[:], ps[:], AF.Exp, scale=1.0 / SCH_A)
            return et

        def emit_av(pair, icq, hh, expt, jlo, jhi, av):
            h = 2 * pair + hh
            for jc in range(jlo, jhi):
                nc.tensor.matmul(
                    av[:],
                    vt_sb[jc][:, h * (DH + 1):(h + 1) * (DH + 1)],
                    expt[jc][:, hh * 512:(hh + 1) * 512],
                    start=(jc == 0), stop=(jc == 7),
                )

        # Normalize chain split into three independently-emittable steps so
        # the DVE->Pool->DVE chain can be spaced a full pipeline stage after
        # its av producer: every step's deps are long-finished when its
        # engine dequeues it (strict-FIFO queues stall on head-of-line
        # dependencies otherwise).
        def emit_recip(av):
            rz = spool.tile([1, 512], f32, tag="rz", name="rz")
            nc.vector.reciprocal(rz[:], av[DH:DH + 1, :])
            return rz

        def emit_bcast(rz):
            bc = spool.tile([DH, 512], f32, tag="bc", name="bc")
            nc.gpsimd.partition_broadcast(bc[:], rz[:])
            return bc

        def emit_mult(pair, icq, hh, av, bc):
            dst = otn_sb[pair][hh * DH:(hh + 1) * DH,
                               icq * 512:(icq + 1) * 512]
            nc.vector.tensor_tensor(dst, av[0:DH, :], bc[:], ALU.mult)

        # 3-deep software pipeline over stages s = (pair, icq):
        #   stage s emission: sims+exps(s) | AVs(s-1) | normalize(s-LAG-1)
        # With PSUM3 the normalize lag drops to the same slot as the AVs
        # (av tiles live one slot, 2 psum bufs suffice, and the freed 2
        # banks give the sims a third psum buffer).
        LAG = 1 if PSUM3 else 2
        stages = [(pair, icq) for pair in range(4) for icq in range(4)]
        N = len(stages)
        av_of = {}     # s -> (avA, avB)
        rz_of = {}     # s -> (rzA, rzB)
        bc_of = {}     # s -> (bcA, bcB)
        expt_of = {}   # s -> {jc: tile}

        def emit_stage(s):
            """Emit one pipeline slot: interleave blocks so no engine queue
            head-of-line blocks on a same-slot dependency."""
            cur = stages[s] if s < N else None
            prv = stages[s - 1] if 1 <= s <= N else None
            o = s - LAG  # stage being normalized this slot
            old = stages[o] if 0 <= o < N else None
            if cur is not None:
                expt_of[s] = {}
            if prv is not None:
                avA = psAv.tile([DH + 1, 512], f32, tag="av", name="avA")
                avB = psAv.tile([DH + 1, 512], f32, tag="av", name="avB")
                av_of[s - 1] = (avA, avB)

            def sim2(j0):
                if cur is not None:
                    expt_of[s][j0] = emit_sim_exp(s, *cur, j0)
                    expt_of[s][j0 + 1] = emit_sim_exp(s, *cur, j0 + 1)

            sim2(0)
            if s == 0:
                emit_vt2(0)
            if prv is not None:
                emit_av(*prv, 0, expt_of[s - 1], 0, 4, av_of[s - 1][0])
            if old is not None and LAG == 2:
                rz_of[o] = (emit_recip(av_of[o][0]),
                            emit_recip(av_of[o][1]))
            sim2(2)
            if s == 0:
                emit_vt2(2)
            if prv is not None:
                emit_av(*prv, 0, expt_of[s - 1], 4, 8, av_of[s - 1][0])
            if old is not None:
                if LAG == 2:
                    bc_of[o] = (emit_bcast(rz_of[o][0]),
                                emit_bcast(rz_of[o][1]))
                else:
                    rz_of[o] = (emit_recip(av_of[o][0]), None)
            sim2(4)
            if s == 0:
                emit_vt2(4)
            if STAGGER and 1 <= s <= 9:
                fn, args = stagger[s - 1]
                fn(*args)
            if prv is not None:
                emit_av(*prv, 1, expt_of[s - 1], 0, 4, av_of[s - 1][1])
            if old is not None and LAG == 1:
                bc_of[o] = (emit_bcast(rz_of[o][0]), None)
            sim2(6)
            if s == 0:
                emit_vt2(6)
            if prv is not None:
                emit_av(*prv, 1, expt_of[s - 1], 4, 8, av_of[s - 1][1])
            if old is not None and LAG == 1:
                rz_of[o] = (rz_of[o][0], emit_recip(av_of[o][1]))
                bc_of[o] = (bc_of[o][0], emit_bcast(rz_of[o][1]))
            if old is not None:
                emit_mult(*old, 0, av_of[o][0], bc_of[o][0])
                emit_mult(*old, 1, av_of[o][1], bc_of[o][1])
                del av_of[o], expt_of[o]
            # output projection rides the pipeline tail: y chunk icq only
            # needs every pair's otn[:, icq], whose last mult is emitted at
            # stage 12+LAG+icq, so chunks stream out during the drain
            # stages instead of serializing after them.
            if s - 12 - LAG >= 0 and (icy := s - 12 - LAG) <= 3:
                emit_y(icy)
            # last slot of the LAG=2 schedule: fold the final normalize +
            # y chunk in instead of paying a whole extra drain slot.
            if LAG == 2 and s == N:
                rzl = (emit_recip(av_of[N - 1][0]),
                       emit_recip(av_of[N - 1][1]))
                bcl = (emit_bcast(rzl[0]), emit_bcast(rzl[1]))
                emit_mult(*stages[N - 1], 0, av_of[N - 1][0], bcl[0])
                emit_mult(*stages[N - 1], 1, av_of[N - 1][1], bcl[1])
                emit_y(3)

        def emit_y(icq):
            for mc in range(2):
                ps = psSim.tile([P, 1024], f32, tag="sim", name="yp")
                for pair in range(4):
                    nc.tensor.matmul(
                        ps[:, 0:512],
                        wo_sb[pair][:, mc * P:(mc + 1) * P],
                        otn_sb[pair][:, icq * 512:(icq + 1) * 512],
                        start=(pair == 0), stop=(pair == 3),
                    )
                nc.vector.tensor_scalar(
                    y_sb[mc][:, icq * 512:(icq + 1) * 512],
                    ps[:, 0:512], bo_sb[mc][:], None, ALU.add)
                nc.sync.dma_start(
                    y_d[mc * P:(mc + 1) * P, icq * 512:(icq + 1) * 512],
                    y_sb[mc][:, icq * 512:(icq + 1) * 512])

        y_sb = [wpool.tile([P, IC], f32, tag=f"y{mc}", name=f"y{mc}")
                for mc in range(2)]

        emit_proj_k(0)
        emit_proj_q(0, 0)
        emit_proj_q(0, 1)
        if not STAGGER:
            # all remaining projections in the head; both psum-drain
            # engines are idle here, so alternate the copies across them
            for i, (fn, args) in enumerate(stagger):
                fn(*args, copy_on_act=(i % 2 == 0))
        for s in range(N + 1):
            emit_stage(s)

    with tile.TileContext(nc) as tc:
        with (
            tc.tile_pool(name="wpool", bufs=1) as wpool,
            tc.tile_pool(name="qpool", bufs=1) as qpool,
            tc.tile_pool(name="kpool", bufs=1) as kpool,
            tc.tile_pool(name="vpool", bufs=1) as vpool,
            tc.tile_pool(name="epool", bufs=1) as epool,
            tc.tile_pool(name="opool", bufs=1) as opool,
            tc.tile_pool(name="spool", bufs=4) as spool,
            tc.tile_pool(name="psSim", bufs=(3 if PSUM3 else 2),
                         space="PSUM") as psSim,
            tc.tile_pool(name="psAv", bufs=(2 if PSUM3 else 4),
                         space="PSUM") as psAv,
        ):
            pools = (wpool, qpool, kpool, vpool, epool, opool, spool,
                     psSim, psAv)
            if reps == 1:
                body(nc, tc, pools)
            else:
                with tc.For_i(0, reps, 1):
                    body(nc, tc, pools)

    nc.compile()
    nc.m = get_hw_module(nc.m)
    return nc


def _shard_inputs(x, a, Wq, Wkv, Wout, bout):
    xf = np.ascontiguousarray(x.reshape(B, CQ, HW), dtype=np.float32)
    af = np.ascontiguousarray(a.reshape(B, CKV, NJ), dtype=np.float32)
    # attention scale and the Schraudolph exp scale both fold into Wq
    wq = np.ascontiguousarray((Wq * (DH ** -0.5 * SCH_A)).T, dtype=np.float32)
    wk = np.ascontiguousarray(Wkv[:HID].T, dtype=np.float32)
    wv = np.ascontiguousarray(Wkv[HID:].T, dtype=np.float32)
    wo = np.ascontiguousarray(Wout.T, dtype=np.float32)
    bo = np.ascontiguousarray(bout.reshape(CQ, 1), dtype=np.float32)
    in_maps = []
    for c in range(8):
        b, half = c // 2, c % 2
        in_maps.append({
            "x": np.ascontiguousarray(xf[b][:, half * IC:(half + 1) * IC]),
            "a": af[b],
            "wq": wq, "wk": wk, "wv": wv, "wo": wo, "bo": bo,
            "ones": np.ones((P, HEADS), dtype=np.float32),
        })
    return in_maps


def _get_runner():
    global _RUNNER
    if _RUNNER is None:
        _RUNNER = _build_nc()
    return _RUNNER


_JIT = None


def _make_jit(nc):
    """Build a sharded PJRT callable for a compiled nc."""
    import jax
    import concourse.mybir as mybir
    from jax.sharding import Mesh, PartitionSpec
    from jax.experimental.shard_map import shard_map
    from concourse.bass2jax import (
        _bass_exec_p, install_neuronx_cc_hook, partition_id_tensor)

    install_neuronx_cc_hook()
    partition_name = (
        nc.partition_id_tensor.name if nc.partition_id_tensor else None)
    in_names, out_names, out_avals, zero_outs = [], [], [], []
    for alloc in nc.m.functions[0].allocations:
        if not isinstance(alloc, mybir.MemoryLocationSet):
            continue
        name = alloc.memorylocations[0].name
        if alloc.kind == "ExternalInput":
            if name != partition_name:
                in_names.append(name)
        elif alloc.kind == "ExternalOutput":
            shape = tuple(alloc.tensor_shape)
            dtype = mybir.dt.np(alloc.dtype)
            out_names.append(name)
            out_avals.append(jax.core.ShapedArray(shape, dtype))
            zero_outs.append((shape, dtype))
    n_params = len(in_names)
    all_in_names = list(in_names) + list(out_names)
    if partition_name is not None:
        all_in_names.append(partition_name)

    def _body(*args):
        operands = list(args)
        if partition_name is not None:
            operands.append(partition_id_tensor())
        outs = _bass_exec_p.bind(
            *operands,
            out_avals=tuple(out_avals),
            in_names=tuple(all_in_names),
            out_names=tuple(out_names),
            lowering_input_output_aliases=(),
            sim_require_finite=True,
            sim_require_nnan=True,
            nc=nc,
        )
        return tuple(outs)

    devices = jax.devices()[:8]
    mesh = Mesh(np.asarray(devices), ("core",))
    in_specs = (PartitionSpec("core"),) * (n_params + len(out_names))
    out_specs = (PartitionSpec("core"),) * len(out_names)
    sharded = jax.jit(
        shard_map(_body, mesh=mesh, in_specs=in_specs, out_specs=out_specs,
                  check_rep=False),
        keep_unused=True)
    return (sharded, in_names, out_names, out_avals, zero_outs)


def _get_jit():
    global _JIT
    if _JIT is None:
        _JIT = _make_jit(_get_runner())
    return _JIT


_DEV_CACHE = {"fp": None, "dev_in": None, "dev_zeros": None}


def _stage_inputs(concat_in, zero_outs):
    """device_put inputs once; reuse when the same bytes are passed again."""
    import jax
    import zlib
    fp = tuple(zlib.adler32(a.tobytes()) for a in concat_in)
    if _DEV_CACHE["fp"] != fp or _DEV_CACHE["dev_in"] is None:
        _DEV_CACHE["dev_in"] = [jax.device_put(a) for a in concat_in]
        _DEV_CACHE["fp"] = fp
    if _DEV_CACHE["dev_zeros"] is None:
        _DEV_CACHE["dev_zeros"] = [
            jax.device_put(np.zeros((8 * s[0], *s[1:]), d))
            for (s, d) in zero_outs
        ]
    return _DEV_CACHE["dev_in"], _DEV_CACHE["dev_zeros"]


def run_sharded(in_maps):
    """Run the SPMD kernel; returns list of per-core output dicts."""
    sharded, in_names, out_names, out_avals, zero_outs = _get_jit()
    concat_in = [
        np.ascontiguousarray(
            np.concatenate([np.asarray(m[name]) for m in in_maps], axis=0))
        for name in in_names
    ]
    dev_in, dev_zeros = _stage_inputs(concat_in, zero_outs)
    out_arrs = sharded(*dev_in, *dev_zeros)
    return [
        {name: np.asarray(out_arrs[i]).reshape(8, *out_avals[i].shape)[c]
         for i, name in enumerate(out_names)}
        for c in range(8)
    ]


def run_staged():
    """Re-run with already-staged device inputs (timing helper)."""
    sharded, in_names, out_names, out_avals, zero_outs = _get_jit()
    out = sharded(*_DEV_CACHE["dev_in"], *_DEV_CACHE["dev_zeros"])
    for o in out:
        o.block_until_ready()
    return out


def kernel(x, a, Wq, Wkv, Wout, bout):
    in_maps = _shard_inputs(
        np.asarray(x), np.asarray(a), np.asarray(Wq), np.asarray(Wkv),
        np.asarray(Wout), np.asarray(bout))
    results = run_sharded(in_maps)
    y = np.empty((B, CQ, HW), dtype=np.float32)
    for c in range(8):
        b, half = c // 2, c % 2
        y[b][:, half * IC:(half + 1) * IC] = results[c]["y"]
    return y.reshape(B, CQ, 64, 64)


# revision 19
# speedup vs baseline: 1.3533x; 1.3533x over previous
"""CrossAttention Trainium2 Bass kernel.

Problem: x[4,256,64,64], a[4,256,32,32], Wq[512,256], Wkv[1024,256],
Wout[256,512], bout[256] -> y[4,256,64,64]  (8 heads, dim_head 64).

Sharding: 8 cores = (batch b in 0..3) x (query-half in 0..1). Each core
computes all 8 heads for a [256, 2048] slice of x (2048 query positions)
against the full [256, 1024] kv field of its batch, and produces the
complete [256, 2048] output slice (no cross-core reduction needed).

Device-side math per core (matmuls in float32r):
  Q  = (0.125*Wq)^T.T @ X      [512, 2048]   (scale folded into Wq on host)
  K  = Wk^T.T @ A              [512, 1024]
  VT = A-chunks.T @ Wv^T       [1024, 520]   (j on partitions, per-head ones
                                              column appended -> Z row)
  simT[j,i] per head pair: two row-tiled K=64 matmuls (tile_position (0,0)
      and (64,0)) write head A to psum cols 0:512 and head B to 512:1024.
  expT: 6 of 8 j-chunks exponentiate on the ACT engine (true exp, bf16 out);
      2 chunks ride the DVE as a Schraudolph add in the bf16 bit domain
      (exp(x) ~= bitcast_bf16(int16(x*A + B)); x*A folded into Wq).
  AV: av[65, i] = vt_aug.T @ expT accumulated over j-chunks; row 64 = Z.
  otn = av * (1/Z) via DVE reciprocal + gpsimd broadcast + DVE mult
  Y  = Wout^T.T @ otn accumulated over head pairs in PSUM, + bout, at end.

Schedule: the PE's total work (~311k cycles ~= 130us) is the hard floor
(sim matmuls run at half array utilization because d=64; fp8 DoubleRow
cannot fix that, and fp8 AV fails the 2e-2 accuracy budget - measured).
So the schedule keeps the PE saturated end to end:
  - inputs stream on 5 DGE rings, ordered by first use; ACT's exp table
    preloads behind the DMA head.
  - pair-0 K/Q project first; stage 0's empty AV slots run the VT
    projections (own PSUM rotation in psAv, no sim-psum contention);
    pairs 1-3 K/Q spread one chunk per stage over stages 1-9.
  - per stage 6 exps on ACT (6.2us) + 2 on DVE (2.4us + 2.6us normalize
    work) both sit under the PE's 6.83us/stage.
"""

import numpy as np

HEADS = 8
DH = 64
HID = 512
CQ = 256
CKV = 256
B = 4
HW = 4096
IC = 2048  # query positions per core
NJ = 1024  # kv positions
P = 128

# Schraudolph exp in the bf16 bit domain:
#   exp(x) ~= bitcast_bf16(int16(x * A + B)), max rel err ~3.4%
SCH_A = 184.6650390625      # 2^7 / ln 2
SCH_B = 16250.66            # 127 * 2^7 - c_adj (robust to trunc/round)

import os as _os
# Which of the 8 exp chunks per stage run on the DVE as a Schraudolph add.
# Default NONE: on real HW the DVE tensor_scalar reading PSUM runs far
# below the cost model (a 2-chunk split measured +37us vs all-ACT), so all
# exps stay on the ACT engine.
DVE_JC = tuple(int(c) for c in _os.environ.get("KRN_DVE_JC", ""))
COPY_ENG = _os.environ.get("KRN_COPY", "dve")     # proj psum-drain engine
STAGGER = int(_os.environ.get("KRN_STAGGER", "0"))  # pairs 1-3 K/Q in-stage
# PSUM3 (3 sim-psum bufs / 1-slot normalize / 2 av bufs) helps in CoreSim
# but regresses on HW: the 1-slot normalize puts the Pool broadcast onto
# the per-stage av-buffer critical chain. Keep the 2-slot normalize.
PSUM3 = int(_os.environ.get("KRN_PSUM3", "0"))

_RUNNER = None


def _build_nc(reps=1):
    import concourse.bass as bass
    import concourse.mybir as mybir
    from concourse import tile, bacc
    from concourse.bass_interp import get_hw_module

    f32 = mybir.dt.float32
    f32r = mybir.dt.float32r
    bf16 = mybir.dt.bfloat16
    i16 = mybir.dt.int16
    AF = mybir.ActivationFunctionType
    ALU = mybir.AluOpType

    nc = bacc.Bacc("TRN2", target_bir_lowering=False, debug=False, num_devices=8)

    x_d = nc.dram_tensor("x", [CQ, IC], f32r, kind="ExternalInput")
    a_d = nc.dram_tensor("a", [CKV, NJ], f32r, kind="ExternalInput")
    wq_d = nc.dram_tensor("wq", [CQ, HID], f32r, kind="ExternalInput")
    wk_d = nc.dram_tensor("wk", [CKV, HID], f32r, kind="ExternalInput")
    wv_d = nc.dram_tensor("wv", [CKV, HID], f32r, kind="ExternalInput")
    wo_d = nc.dram_tensor("wo", [HID, CQ], f32r, kind="ExternalInput")
    bo_d = nc.dram_tensor("bo", [CQ, 1], f32, kind="ExternalInput")
    ones_d = nc.dram_tensor("ones", [P, HEADS], f32r, kind="ExternalInput")
    y_d = nc.dram_tensor("y", [CQ, IC], f32, kind="ExternalOutput")

    def body(nc, tc, pools):
        wpool, qpool, kpool, vpool, epool, opool, spool, psSim, psAv = pools

        # ---- ACT exp-table preload: a dummy 1-elem exp issued first makes
        # the 1.3us table load ride the DMA head instead of the first real
        # exp on the critical path.
        sc = spool.tile([1, 2], f32, tag="sc", name="sc")
        nc.vector.memset(sc[:, 0:1], 0.0)
        nc.scalar.activation(sc[0:1, 1:2], sc[0:1, 0:1], AF.Exp)

        # ---- input DMAs on the 3 DGE rings (SP, ACT, GPSIMD), ordered by
        # first use: a (K/VT input) on sync+gpsimd, wk+wq on scalar (K0/Q0
        # weights), then the x halves (Q proj input, gates the pipeline
        # head) round-robin; wv/wo/bo ride behind (not read until VT / the
        # output projection).
        a_sb = [wpool.tile([P, NJ], f32r, tag=f"a{kc}", name=f"a{kc}")
                for kc in range(2)]
        for kc in range(2):
            for half in range(2):
                ring = nc.sync if (kc + half) % 2 == 0 else nc.gpsimd
                ring.dma_start(
                    a_sb[kc][:, half * 512:(half + 1) * 512],
                    a_d[kc * P:(kc + 1) * P, half * 512:(half + 1) * 512])
        # NOTE: a dma_start's descriptor generation runs ON the issuing
        # engine (~0.8us per [128,512] f32 chunk). The scalar ring would
        # bill that to the ACT engine - the exp bottleneck - so every DMA
        # rides sync (SP, idle) or gpsimd (Pool, mostly idle) instead.
        wk_sb = [wpool.tile([P, HID], f32r, tag=f"wk{kc}", name=f"wk{kc}")
                 for kc in range(2)]
        wq_sb = [wpool.tile([P, HID], f32r, tag=f"wq{kc}", name=f"wq{kc}")
                 for kc in range(2)]
        for kc in range(2):
            nc.sync.dma_start(wk_sb[kc][:], wk_d[kc * P:(kc + 1) * P, :])
        for kc in range(2):
            nc.gpsimd.dma_start(wq_sb[kc][:], wq_d[kc * P:(kc + 1) * P, :])
        x_sb = [wpool.tile([P, IC], f32r, tag=f"x{kc}", name=f"x{kc}")
                for kc in range(2)]
        xring = [nc.sync, nc.gpsimd, nc.gpsimd, nc.sync]
        for half in range(2):
            for kc in range(2):
                xring[half * 2 + kc].dma_start(
                    x_sb[kc][:, half * 1024:(half + 1) * 1024],
                    x_d[kc * P:(kc + 1) * P, half * 1024:(half + 1) * 1024])
        wv_sb = []
        for kc in range(2):
            t = wpool.tile([P, HID], f32r, tag=f"wv{kc}", name=f"wv{kc}")
            nc.gpsimd.dma_start(t[:], wv_d[kc * P:(kc + 1) * P, :])
            wv_sb.append(t)
        wo_sb = []
        for pc in range(4):
            t = wpool.tile([P, CQ], f32r, tag=f"wo{pc}", name=f"wo{pc}")
            nc.sync.dma_start(t[:], wo_d[pc * P:(pc + 1) * P, :])
            wo_sb.append(t)
        bo_sb = []
        for mc in range(2):
            t = wpool.tile([P, 1], f32, tag=f"bo{mc}", name=f"bo{mc}")
            nc.gpsimd.dma_start(t[:], bo_d[mc * P:(mc + 1) * P, :])
            bo_sb.append(t)

        # ---- projections ----
        # matmul(out, lhsT, rhs): out = lhsT.T @ rhs.
        q_sb = [qpool.tile([P, IC], f32r, tag=f"q{mc}", name=f"q{mc}")
                for mc in range(4)]
        k_sb = [kpool.tile([P, NJ], f32r, tag=f"k{mc}", name=f"k{mc}")
                for mc in range(4)]
        vt_sb = []

        def emit_proj_k(mc, copy_on_act=(COPY_ENG == "act")):
            ps = psSim.tile([P, 1024], f32, tag="sim", name="psk")
            for half in range(2):
                for kc in range(2):
                    nc.tensor.matmul(
                        ps[:, half * 512:(half + 1) * 512],
                        wk_sb[kc][:, mc * P:(mc + 1) * P],
                        a_sb[kc][:, half * 512:(half + 1) * 512],
                        start=(kc == 0), stop=(kc == 1),
                    )
            # psum-drain copies ride the ACT engine (it has headroom; the
            # DVE carries the exps + normalize and is the busier engine)
            if copy_on_act:
                nc.scalar.activation(k_sb[mc][:], ps[:], AF.Copy)
            else:
                nc.vector.tensor_copy(k_sb[mc][:], ps[:])

        def emit_proj_q(mc, n, copy_on_act=(COPY_ENG == "act")):
            ps = psSim.tile([P, 1024], f32, tag="sim", name="psq")
            for half in range(2):
                for kc in range(2):
                    nc.tensor.matmul(
                        ps[:, half * 512:(half + 1) * 512],
                        wq_sb[kc][:, mc * P:(mc + 1) * P],
                        x_sb[kc][:, n * 1024 + half * 512:
                                  n * 1024 + (half + 1) * 512],
                        start=(kc == 0), stop=(kc == 1),
                    )
            dst = q_sb[mc][:, n * 1024:(n + 1) * 1024]
            if copy_on_act:
                nc.scalar.activation(dst, ps[:], AF.Copy)
            else:
                nc.vector.tensor_copy(dst, ps[:])

        # VT[j, hd] = sum_c a[c, j] wv[c, hd] : [1024, 512], with per-head
        # ones column appended -> vt tiles [128, 520]. Emitted inside stage
        # 0's (empty) AV slots, on the psAv rotation so the sims' psum
        # buffers stay out of the way.
        for jc in range(8):
            vt_sb.append(vpool.tile([P, HEADS * (DH + 1)], bf16,
                                    tag=f"vt{jc}", name=f"vt{jc}"))

        def emit_vt2(j0):
            for jc in (j0, j0 + 1):
                vt = vt_sb[jc]
                ones_dst = vt[:].rearrange(
                    "p (h d) -> p h d", h=HEADS, d=DH + 1)[:, :, DH:DH + 1]
                nc.gpsimd.dma_start(ones_dst, ones_d[:].unsqueeze(-1))
                ps = psAv.tile([P, 512], f32, tag="av", name=f"vtp{jc}")
                for kc in range(2):
                    nc.tensor.matmul(
                        ps[:],
                        a_sb[kc][:, jc * P:(jc + 1) * P],
                        wv_sb[kc][:],
                        start=(kc == 0), stop=(kc == 1),
                    )
                dst = vt[:].rearrange(
                    "p (h d) -> p h d", h=HEADS, d=DH + 1)[:, :, 0:DH]
                src = ps[:].rearrange("p (h d) -> p h d", h=HEADS, d=DH)
                nc.vector.tensor_copy(dst, src)

        # pairs 1-3 K/Q spread one psSim-sized chunk per stage, stages 1-9
        # (pair p is first read at stage 4p, so every chunk lands early).
        stagger = [(emit_proj_k, (1,)), (emit_proj_q, (1, 0)),
                   (emit_proj_q, (1, 1)),
                   (emit_proj_k, (2,)), (emit_proj_q, (2, 0)),
                   (emit_proj_q, (2, 1)),
                   (emit_proj_k, (3,)), (emit_proj_q, (3, 0)),
                   (emit_proj_q, (3, 1))]

        # ---- attention stages ----
        otn_sb = [opool.tile([P, IC], f32r, tag=f"otn{p}", name=f"otn{p}")
                  for p in range(4)]

        def emit_sim_exp(s, pair, icq, jc):
            """Row-tiled sim pair (heads 2*pair, 2*pair+1) + exp."""
            ps = psSim.tile([P, 1024], f32, tag="sim", name="sim")
            nc.tensor.matmul(
                ps[:, 0:512],
                k_sb[pair][0:DH, jc * P:(jc + 1) * P],
                q_sb[pair][0:DH, icq * 512:(icq + 1) * 512],
                start=True, stop=True, tile_position=(0, 0),
            )
            nc.tensor.matmul(
                ps[:, 512:1024],
                k_sb[pair][DH:2 * DH, jc * P:(jc + 1) * P],
                q_sb[pair][DH:2 * DH, icq * 512:(icq + 1) * 512],
                start=True, stop=True, tile_position=(64, 0),
            )
            # expt double-buffered by stage parity: stage s+1's exps must
            # not overwrite the tiles stage s's AV matmuls still read.
            et = epool.tile([P, 1024], bf16, tag=f"e{jc}_{s % 2}",
                            name=f"e{jc}")
            if jc in DVE_JC:
                nc.vector.tensor_scalar(
                    et[:].bitcast(i16), ps[:], SCH_B, None, ALU.add)
            else:
                nc.scalar.activation(et[:], ps[:], AF.Exp, scale=1.0 / SCH_A)
            return et

        def emit_av(pair, icq, hh, expt, jlo, jhi, av):
            h = 2 * pair + hh
            for jc in range(jlo, jhi):
                nc.tensor.matmul(
                    av[:],
                    vt_sb[jc][:, h * (DH + 1):(h + 1) * (DH + 1)],
                    expt[jc][:, hh * 512:(hh + 1) * 512],
                    start=(jc == 0), stop=(jc == 7),
                )

        # Normalize chain split into three independently-emittable steps so
        # the DVE->Pool->DVE chain can be spaced a full pipeline stage after
        # its av producer: every step's deps are long-finished when its
        # engine dequeues it (strict-FIFO queues stall on head-of-line
        # dependencies otherwise).
        def emit_recip(av):
            rz = spool.tile([1, 512], f32, tag="rz", name="rz")
            nc.vector.reciprocal(rz[:], av[DH:DH + 1, :])
            return rz

        def emit_bcast(rz):
            bc = spool.tile([DH, 512], f32, tag="bc", name="bc")
            nc.gpsimd.partition_broadcast(bc[:], rz[:])
            return bc

        def emit_mult(pair, icq, hh, av, bc):
            dst = otn_sb[pair][hh * DH:(hh + 1) * DH,
                               icq * 512:(icq + 1) * 512]
            nc.vector.tensor_tensor(dst, av[0:DH, :], bc[:], ALU.mult)

        # 3-deep software pipeline over stages s = (pair, icq):
        #   stage s emission: sims+exps(s) | AVs(s-1) | normalize(s-LAG-1)
        # With PSUM3 the normalize lag drops to the same slot as the AVs
        # (av tiles live one slot, 2 psum bufs suffice, and the freed 2
        # banks give the sims a third psum buffer).
        LAG = 1 if PSUM3 else 2
        stages = [(pair, icq) for pair in range(4) for icq in range(4)]
        N = len(stages)
        av_of = {}     # s -> (avA, avB)
        rz_of = {}     # s -> (rzA, rzB)
        bc_of = {}     # s -> (bcA, bcB)
        expt_of = {}   # s -> {jc: tile}

        def emit_stage(s):
            """Emit one pipeline slot: interleave blocks so no engine queue
            head-of-line blocks on a same-slot dependency."""
            cur = stages[s] if s < N else None
            prv = stages[s - 1] if 1 <= s <= N else None
            o = s - LAG  # stage being normalized this slot
            old = stages[o] if 0 <= o < N else None
            if cur is not None:
                expt_of[s] = {}
            if prv is not None:
                avA = psAv.tile([DH + 1, 512], f32, tag="av", name="avA")
                avB = psAv.tile([DH + 1, 512], f32, tag="av", name="avB")
                av_of[s - 1] = (avA, avB)

            def sim2(j0):
                if cur is not None:
                    expt_of[s][j0] = emit_sim_exp(s, *cur, j0)
                    expt_of[s][j0 + 1] = emit_sim_exp(s, *cur, j0 + 1)

            sim2(0)
            if s == 0:
                emit_vt2(0)
            if prv is not None:
                emit_av(*prv, 0, expt_of[s - 1], 0, 4, av_of[s - 1][0])
            if old is not None and LAG == 2:
                rz_of[o] = (emit_recip(av_of[o][0]),
                            emit_recip(av_of[o][1]))
            sim2(2)
            if s == 0:
                emit_vt2(2)
            if prv is not None:
                emit_av(*prv, 0, expt_of[s - 1], 4, 8, av_of[s - 1][0])
            if old is not None:
                if LAG == 2:
                    bc_of[o] = (emit_bcast(rz_of[o][0]),
                                emit_bcast(rz_of[o][1]))
                else:
                    rz_of[o] = (emit_recip(av_of[o][0]), None)
            sim2(4)
            if s == 0:
                emit_vt2(4)
            if STAGGER and 1 <= s <= 9:
                fn, args = stagger[s - 1]
                fn(*args)
            if prv is not None:
                emit_av(*prv, 1, expt_of[s - 1], 0, 4, av_of[s - 1][1])
            if old is not None and LAG == 1:
                bc_of[o] = (emit_bcast(rz_of[o][0]), None)
            sim2(6)
            if s == 0:
                emit_vt2(6)
            if prv is not None:
                emit_av(*prv, 1, expt_of[s - 1], 4, 8, av_of[s - 1][1])
            if old is not None and LAG == 1:
                rz_of[o] = (rz_of[o][0], emit_recip(av_of[o][1]))
                bc_of[o] = (bc_of[o][0], emit_bcast(rz_of[o][1]))
            if old is not None:
                emit_mult(*old, 0, av_of[o][0], bc_of[o][0])
                emit_mult(*old, 1, av_of[o][1], bc_of[o][1])
                del av_of[o], expt_of[o]
            # output projection rides the pipeline tail: y chunk icq only
            # needs every pair's otn[:, icq], whose last mult is emitted at
            # stage 12+LAG+icq, so chunks stream out during the drain
            # stages instead of serializing after them.
            if s - 12 - LAG >= 0 and (icy := s - 12 - LAG) <= 3:
                emit_y(icy)
            # last slot of the LAG=2 schedule: fold the final normalize +
            # y chunk in instead of paying a whole extra drain slot.
            if LAG == 2 and s == N:
                rzl = (emit_recip(av_of[N - 1][0]),
                       emit_recip(av_of[N - 1][1]))
                bcl = (emit_bcast(rzl[0]), emit_bcast(rzl[1]))
                emit_mult(*stages[N - 1], 0, av_of[N - 1][0], bcl[0])
                emit_mult(*stages[N - 1], 1, av_of[N - 1][1], bcl[1])
                emit_y(3)

        def emit_y(icq):
            for mc in range(2):
                ps = psSim.tile([P, 1024], f32, tag="sim", name="yp")
                for pair in range(4):
                    nc.tensor.matmul(
                        ps[:, 0:512],
                        wo_sb[pair][:, mc * P:(mc + 1) * P],
                        otn_sb[pair][:, icq * 512:(icq + 1) * 512],
                        start=(pair == 0), stop=(pair == 3),
                    )
                nc.vector.tensor_scalar(
                    y_sb[mc][:, icq * 512:(icq + 1) * 512],
                    ps[:, 0:512], bo_sb[mc][:], None, ALU.add)
                nc.sync.dma_start(
                    y_d[mc * P:(mc + 1) * P, icq * 512:(icq + 1) * 512],
                    y_sb[mc][:, icq * 512:(icq + 1) * 512])

        y_sb = [wpool.tile([P, IC], f32, tag=f"y{mc}", name=f"y{mc}")
                for mc in range(2)]

        emit_proj_k(0)
        emit_proj_q(0, 0)
        emit_proj_q(0, 1)
        if not STAGGER:
            # all remaining projections in the head; both psum-drain
            # engines are idle here, so alternate the copies across them
            for i, (fn, args) in enumerate(stagger):
                fn(*args, copy_on_act=(i % 2 == 0))
        for s in range(N + 1):
            emit_stage(s)

    with tile.TileContext(nc) as tc:
        with (
            tc.tile_pool(name="wpool", bufs=1) as wpool,
            tc.tile_pool(name="qpool", bufs=1) as qpool,
            tc.tile_pool(name="kpool", bufs=1) as kpool,
            tc.tile_pool(name="vpool", bufs=1) as vpool,
            tc.tile_pool(name="epool", bufs=1) as epool,
            tc.tile_pool(name="opool", bufs=1) as opool,
            tc.tile_pool(name="spool", bufs=4) as spool,
            tc.tile_pool(name="psSim", bufs=(3 if PSUM3 else 2),
                         space="PSUM") as psSim,
            tc.tile_pool(name="psAv", bufs=(2 if PSUM3 else 4),
                         space="PSUM") as psAv,
        ):
            pools = (wpool, qpool, kpool, vpool, epool, opool, spool,
                     psSim, psAv)
            if reps == 1:
                body(nc, tc, pools)
            else:
                with tc.For_i(0, reps, 1):
                    body(nc, tc, pools)

    nc.compile()
    nc.m = get_hw_module(nc.m)
    return nc


def _shard_inputs(x, a, Wq, Wkv, Wout, bout):
    xf = np.ascontiguousarray(x.reshape(B, CQ, HW), dtype=np.float32)
    af = np.ascontiguousarray(a.reshape(B, CKV, NJ), dtype=np.float32)
    # attention scale and the Schraudolph exp scale both fold into Wq
    wq = np.ascontiguousarray((Wq * (DH ** -0.5 * SCH_A)).T, dtype=np.float32)
    wk = np.ascontiguousarray(Wkv[:HID].T, dtype=np.float32)
    wv = np.ascontiguousarray(Wkv[HID:].T, dtype=np.float32)
    wo = np.ascontiguousarray(Wout.T, dtype=np.float32)
    bo = np.ascontiguousarray(bout.reshape(CQ, 1), dtype=np.float32)
    in_maps = []
    for c in range(8):
        b, half = c // 2, c % 2
        in_maps.append({
            "x": np.ascontiguousarray(xf[b][:, half * IC:(half + 1) * IC]),
            "a": af[b],
            "wq": wq, "wk": wk, "wv": wv, "wo": wo, "bo": bo,
            "ones": np.ones((P, HEADS), dtype=np.float32),
        })
    return in_maps


def _get_runner():
    global _RUNNER
    if _RUNNER is None:
        _RUNNER = _build_nc()
    return _RUNNER


_JIT = None


def _make_jit(nc):
    """Build a sharded PJRT callable for a compiled nc."""
    import jax
    import concourse.mybir as mybir
    from jax.sharding import Mesh, PartitionSpec
    from jax.experimental.shard_map import shard_map
    from concourse.bass2jax import (
        _bass_exec_p, install_neuronx_cc_hook, partition_id_tensor)

    install_neuronx_cc_hook()
    partition_name = (
        nc.partition_id_tensor.name if nc.partition_id_tensor else None)
    in_names, out_names, out_avals, zero_outs = [], [], [], []
    for alloc in nc.m.functions[0].allocations:
        if not isinstance(alloc, mybir.MemoryLocationSet):
            continue
        name = alloc.memorylocations[0].name
        if alloc.kind == "ExternalInput":
            if name != partition_name:
                in_names.append(name)
        elif alloc.kind == "ExternalOutput":
            shape = tuple(alloc.tensor_shape)
            dtype = mybir.dt.np(alloc.dtype)
            out_names.append(name)
            out_avals.append(jax.core.ShapedArray(shape, dtype))
            zero_outs.append((shape, dtype))
    n_params = len(in_names)
    all_in_names = list(in_names) + list(out_names)
    if partition_name is not None:
        all_in_names.append(partition_name)

    def _body(*args):
        operands = list(args)
        if partition_name is not None:
            operands.append(partition_id_tensor())
        outs = _bass_exec_p.bind(
            *operands,
            out_avals=tuple(out_avals),
            in_names=tuple(all_in_names),
            out_names=tuple(out_names),
            lowering_input_output_aliases=(),
            sim_require_finite=True,
            sim_require_nnan=True,
            nc=nc,
        )
        return tuple(outs)

    devices = jax.devices()[:8]
    mesh = Mesh(np.asarray(devices), ("core",))
    in_specs = (PartitionSpec("core"),) * (n_params + len(out_names))
    out_specs = (PartitionSpec("core"),) * len(out_names)
    sharded = jax.jit(
        shard_map(_body, mesh=mesh, in_specs=in_specs, out_specs=out_specs,
                  check_rep=False),
        keep_unused=True)
    return (sharded, in_names, out_names, out_avals, zero_outs)


def _get_jit():
    global _JIT
    if _JIT is None:
        _JIT = _make_jit(_get_runner())
    return _JIT


_DEV_CACHE = {"fp": None, "dev_in": None, "dev_zeros": None}


def _stage_inputs(concat_in, zero_outs):
    """device_put inputs once; reuse when the same bytes are passed again."""
    import jax
    import zlib
    fp = tuple(zlib.adler32(a.tobytes()) for a in concat_in)
    if _DEV_CACHE["fp"] != fp or _DEV_CACHE["dev_in"] is None:
        _DEV_CACHE["dev_in"] = [jax.device_put(a) for a in concat_in]
        _DEV_CACHE["fp"] = fp
    if _DEV_CACHE["dev_zeros"] is None:
        _DEV_CACHE["dev_zeros"] = [
            jax.device_put(np.zeros((8 * s[0], *s[1:]), d))
            for (s, d) in zero_outs
        ]
    return _DEV_CACHE["dev_in"], _DEV_CACHE["dev_zeros"]


def run_sharded(in_maps):
    """Run the SPMD kernel; returns list of per-core output dicts."""
    sharded, in_names, out_names, out_avals, zero_outs = _get_jit()
    concat_in = [
        np.ascontiguousarray(
            np.concatenate([np.asarray(m[name]) for m in in_maps], axis=0))
        for name in in_names
    ]
    dev_in, dev_zeros = _stage_inputs(concat_in, zero_outs)
    out_arrs = sharded(*dev_in, *dev_zeros)
    return [
        {name: np.asarray(out_arrs[i]).reshape(8, *out_avals[i].shape)[c]
         for i, name in enumerate(out_names)}
        for c in range(8)
    ]


def run_staged():
    """Re-run with already-staged device inputs (timing helper)."""
    sharded, in_names, out_names, out_avals, zero_outs = _get_jit()
    out = sharded(*_DEV_CACHE["dev_in"], *_DEV_CACHE["dev_zeros"])
    for o in out:
        o.block_until_ready()
    return out


def kernel(x, a, Wq, Wkv, Wout, bout):
    in_maps = _shard_inputs(
        np.asarray(x), np.asarray(a), np.asarray(Wq), np.asarray(Wkv),
        np.asarray(Wout), np.asarray(bout))
    results = run_sharded(in_maps)
    y = np.empty((B, CQ, HW), dtype=np.float32)
    for c in range(8):
        b, half = c // 2, c % 2
        y[b][:, half * IC:(half + 1) * IC] = results[c]["y"]
    return y.reshape(B, CQ, 64, 64)


# revision 20
# speedup vs baseline: 1.5645x; 1.1560x over previous
"""CrossAttention Trainium2 Bass kernel.

Problem: x[4,256,64,64], a[4,256,32,32], Wq[512,256], Wkv[1024,256],
Wout[256,512], bout[256] -> y[4,256,64,64]  (8 heads, dim_head 64).

Sharding: 8 cores = (batch b in 0..3) x (query-half in 0..1). Each core
computes all 8 heads for a [256, 2048] slice of x (2048 query positions)
against the full [256, 1024] kv field of its batch, and produces the
complete [256, 2048] output slice (no cross-core reduction needed).

Device-side math per core (matmuls in float32r):
  Q  = (0.125*Wq)^T.T @ X      [512, 2048]   (scale folded into Wq on host)
  K  = Wk^T.T @ A              [512, 1024]
  VT = A-chunks.T @ Wv^T       [1024, 520]   (j on partitions, per-head ones
                                              column appended -> Z row)
  simT[j,i] per head pair: two row-tiled K=64 matmuls (tile_position (0,0)
      and (64,0)) write head A to psum cols 0:512 and head B to 512:1024.
  expT: 6 of 8 j-chunks exponentiate on the ACT engine (true exp, bf16 out);
      2 chunks ride the DVE as a Schraudolph add in the bf16 bit domain
      (exp(x) ~= bitcast_bf16(int16(x*A + B)); x*A folded into Wq).
  AV: av[65, i] = vt_aug.T @ expT accumulated over j-chunks; row 64 = Z.
  otn = av * (1/Z) via DVE reciprocal + gpsimd broadcast + DVE mult
  Y  = Wout^T.T @ otn accumulated over head pairs in PSUM, + bout, at end.

Schedule: the PE's total work (~311k cycles ~= 130us) is the hard floor
(sim matmuls run at half array utilization because d=64; fp8 DoubleRow
cannot fix that, and fp8 AV fails the 2e-2 accuracy budget - measured).
So the schedule keeps the PE saturated end to end:
  - inputs stream on 5 DGE rings, ordered by first use; ACT's exp table
    preloads behind the DMA head.
  - pair-0 K/Q project first; stage 0's empty AV slots run the VT
    projections (own PSUM rotation in psAv, no sim-psum contention);
    pairs 1-3 K/Q spread one chunk per stage over stages 1-9.
  - per stage 6 exps on ACT (6.2us) + 2 on DVE (2.4us + 2.6us normalize
    work) both sit under the PE's 6.83us/stage.
"""

import numpy as np

HEADS = 8
DH = 64
HID = 512
CQ = 256
CKV = 256
B = 4
HW = 4096
IC = 2048  # query positions per core
NJ = 1024  # kv positions
P = 128

# Schraudolph exp in the bf16 bit domain:
#   exp(x) ~= bitcast_bf16(int16(x * A + B)), max rel err ~3.4%
SCH_A = 184.6650390625      # 2^7 / ln 2
SCH_B = 16250.66            # 127 * 2^7 - c_adj (robust to trunc/round)

import os as _os
# Which of the 8 exp chunks per stage run on the DVE as a Schraudolph add.
# Default NONE: on real HW the DVE tensor_scalar reading PSUM runs far
# below the cost model (a 2-chunk split measured +37us vs all-ACT), so all
# exps stay on the ACT engine.
DVE_JC = tuple(int(c) for c in _os.environ.get("KRN_DVE_JC", ""))
COPY_ENG = _os.environ.get("KRN_COPY", "dve")     # proj psum-drain engine
STAGGER = int(_os.environ.get("KRN_STAGGER", "0"))  # pairs 1-3 K/Q in-stage
# PSUM3 (3 sim-psum bufs / 1-slot normalize / 2 av bufs) helps in CoreSim
# but regresses on HW: the 1-slot normalize puts the Pool broadcast onto
# the per-stage av-buffer critical chain. Keep the 2-slot normalize.
PSUM3 = int(_os.environ.get("KRN_PSUM3", "0"))

_RUNNER = None


def _build_nc(reps=1):
    import concourse.bass as bass
    import concourse.mybir as mybir
    from concourse import tile, bacc
    from concourse.bass_interp import get_hw_module

    f32 = mybir.dt.float32
    f32r = mybir.dt.float32r
    bf16 = mybir.dt.bfloat16
    i16 = mybir.dt.int16
    AF = mybir.ActivationFunctionType
    ALU = mybir.AluOpType

    nc = bacc.Bacc("TRN2", target_bir_lowering=False, debug=False, num_devices=8)

    x_d = nc.dram_tensor("x", [CQ, IC], f32r, kind="ExternalInput")
    a_d = nc.dram_tensor("a", [CKV, NJ], f32r, kind="ExternalInput")
    wq_d = nc.dram_tensor("wq", [CQ, HID], f32r, kind="ExternalInput")
    wk_d = nc.dram_tensor("wk", [CKV, HID], f32r, kind="ExternalInput")
    wv_d = nc.dram_tensor("wv", [CKV, HID], f32r, kind="ExternalInput")
    wo_d = nc.dram_tensor("wo", [HID, CQ], f32r, kind="ExternalInput")
    bo_d = nc.dram_tensor("bo", [CQ, 1], f32, kind="ExternalInput")
    ones_d = nc.dram_tensor("ones", [P, HEADS], f32r, kind="ExternalInput")
    y_d = nc.dram_tensor("y", [CQ, IC], f32, kind="ExternalOutput")

    def body(nc, tc, pools):
        wpool, qpool, kpool, vpool, epool, opool, spool, psSim, psAv = pools

        # ---- ACT exp-table preload: a dummy 1-elem exp issued first makes
        # the 1.3us table load ride the DMA head instead of the first real
        # exp on the critical path.
        sc = spool.tile([1, 2], f32, tag="sc", name="sc")
        nc.vector.memset(sc[:, 0:1], 0.0)
        nc.scalar.activation(sc[0:1, 1:2], sc[0:1, 0:1], AF.Exp)

        # ---- input DMAs on the 3 DGE rings (SP, ACT, GPSIMD), ordered by
        # first use: a (K/VT input) on sync+gpsimd, wk+wq on scalar (K0/Q0
        # weights), then the x halves (Q proj input, gates the pipeline
        # head) round-robin; wv/wo/bo ride behind (not read until VT / the
        # output projection).
        a_sb = [wpool.tile([P, NJ], f32r, tag=f"a{kc}", name=f"a{kc}")
                for kc in range(2)]
        for kc in range(2):
            for half in range(2):
                ring = nc.sync if (kc + half) % 2 == 0 else nc.gpsimd
                ring.dma_start(
                    a_sb[kc][:, half * 512:(half + 1) * 512],
                    a_d[kc * P:(kc + 1) * P, half * 512:(half + 1) * 512])
        # NOTE: a dma_start's descriptor generation runs ON the issuing
        # engine (~0.8us per [128,512] f32 chunk). The scalar ring would
        # bill that to the ACT engine - the exp bottleneck - so every DMA
        # rides sync (SP, idle) or gpsimd (Pool, mostly idle) instead.
        wk_sb = [wpool.tile([P, HID], f32r, tag=f"wk{kc}", name=f"wk{kc}")
                 for kc in range(2)]
        wq_sb = [wpool.tile([P, HID], f32r, tag=f"wq{kc}", name=f"wq{kc}")
                 for kc in range(2)]
        for kc in range(2):
            nc.sync.dma_start(wk_sb[kc][:], wk_d[kc * P:(kc + 1) * P, :])
        for kc in range(2):
            nc.gpsimd.dma_start(wq_sb[kc][:], wq_d[kc * P:(kc + 1) * P, :])
        x_sb = [wpool.tile([P, IC], f32r, tag=f"x{kc}", name=f"x{kc}")
                for kc in range(2)]
        xring = [nc.sync, nc.gpsimd, nc.gpsimd, nc.sync]
        for half in range(2):
            for kc in range(2):
                xring[half * 2 + kc].dma_start(
                    x_sb[kc][:, half * 1024:(half + 1) * 1024],
                    x_d[kc * P:(kc + 1) * P, half * 1024:(half + 1) * 1024])
        wv_sb = []
        for kc in range(2):
            t = wpool.tile([P, HID], f32r, tag=f"wv{kc}", name=f"wv{kc}")
            nc.gpsimd.dma_start(t[:], wv_d[kc * P:(kc + 1) * P, :])
            wv_sb.append(t)
        wo_sb = []
        for pc in range(4):
            t = wpool.tile([P, CQ], f32r, tag=f"wo{pc}", name=f"wo{pc}")
            nc.sync.dma_start(t[:], wo_d[pc * P:(pc + 1) * P, :])
            wo_sb.append(t)
        bo_sb = []
        for mc in range(2):
            t = wpool.tile([P, 1], f32, tag=f"bo{mc}", name=f"bo{mc}")
            nc.gpsimd.dma_start(t[:], bo_d[mc * P:(mc + 1) * P, :])
            bo_sb.append(t)

        # ---- projections ----
        # matmul(out, lhsT, rhs): out = lhsT.T @ rhs.
        q_sb = [qpool.tile([P, IC], f32r, tag=f"q{mc}", name=f"q{mc}")
                for mc in range(4)]
        k_sb = [kpool.tile([P, NJ], f32r, tag=f"k{mc}", name=f"k{mc}")
                for mc in range(4)]
        vt_sb = []

        def emit_proj_k(mc, copy_on_act=(COPY_ENG == "act")):
            ps = psSim.tile([P, 1024], f32, tag="sim", name="psk")
            for half in range(2):
                for kc in range(2):
                    nc.tensor.matmul(
                        ps[:, half * 512:(half + 1) * 512],
                        wk_sb[kc][:, mc * P:(mc + 1) * P],
                        a_sb[kc][:, half * 512:(half + 1) * 512],
                        start=(kc == 0), stop=(kc == 1),
                    )
            # psum-drain copies ride the ACT engine (it has headroom; the
            # DVE carries the exps + normalize and is the busier engine)
            if copy_on_act:
                nc.scalar.activation(k_sb[mc][:], ps[:], AF.Copy)
            else:
                nc.vector.tensor_copy(k_sb[mc][:], ps[:])

        def emit_proj_q(mc, n, copy_on_act=(COPY_ENG == "act")):
            ps = psSim.tile([P, 1024], f32, tag="sim", name="psq")
            for half in range(2):
                for kc in range(2):
                    nc.tensor.matmul(
                        ps[:, half * 512:(half + 1) * 512],
                        wq_sb[kc][:, mc * P:(mc + 1) * P],
                        x_sb[kc][:, n * 1024 + half * 512:
                                  n * 1024 + (half + 1) * 512],
                        start=(kc == 0), stop=(kc == 1),
                    )
            dst = q_sb[mc][:, n * 1024:(n + 1) * 1024]
            if copy_on_act:
                nc.scalar.activation(dst, ps[:], AF.Copy)
            else:
                nc.vector.tensor_copy(dst, ps[:])

        # VT[j, hd] = sum_c a[c, j] wv[c, hd] : [1024, 512], with per-head
        # ones column appended -> vt tiles [128, 520]. Emitted inside stage
        # 0's (empty) AV slots, on the psAv rotation so the sims' psum
        # buffers stay out of the way.
        for jc in range(8):
            vt_sb.append(vpool.tile([P, HEADS * (DH + 1)], bf16,
                                    tag=f"vt{jc}", name=f"vt{jc}"))

        def emit_vt2(j0):
            for jc in (j0, j0 + 1):
                vt = vt_sb[jc]
                ones_dst = vt[:].rearrange(
                    "p (h d) -> p h d", h=HEADS, d=DH + 1)[:, :, DH:DH + 1]
                nc.gpsimd.dma_start(ones_dst, ones_d[:].unsqueeze(-1))
                ps = psAv.tile([P, 512], f32, tag="av", name=f"vtp{jc}")
                for kc in range(2):
                    nc.tensor.matmul(
                        ps[:],
                        a_sb[kc][:, jc * P:(jc + 1) * P],
                        wv_sb[kc][:],
                        start=(kc == 0), stop=(kc == 1),
                    )
                dst = vt[:].rearrange(
                    "p (h d) -> p h d", h=HEADS, d=DH + 1)[:, :, 0:DH]
                src = ps[:].rearrange("p (h d) -> p h d", h=HEADS, d=DH)
                nc.vector.tensor_copy(dst, src)

        # pairs 1-3 K/Q spread one psSim-sized chunk per stage, stages 1-9
        # (pair p is first read at stage 4p, so every chunk lands early).
        stagger = [(emit_proj_k, (1,)), (emit_proj_q, (1, 0)),
                   (emit_proj_q, (1, 1)),
                   (emit_proj_k, (2,)), (emit_proj_q, (2, 0)),
                   (emit_proj_q, (2, 1)),
                   (emit_proj_k, (3,)), (emit_proj_q, (3, 0)),
                   (emit_proj_q, (3, 1))]

        # ---- attention stages ----
        otn_sb = [opool.tile([P, IC], f32r, tag=f"otn{p}", name=f"otn{p}")
                  for p in range(4)]

        def emit_sim_exp(s, pair, icq, jc):
            """Row-tiled sim pair (heads 2*pair, 2*pair+1) + exp."""
            ps = psSim.tile([P, 1024], f32, tag="sim", name="sim")
            nc.tensor.matmul(
                ps[:, 0:512],
                k_sb[pair][0:DH, jc * P:(jc + 1) * P],
                q_sb[pair][0:DH, icq * 512:(icq + 1) * 512],
                start=True, stop=True, tile_position=(0, 0),
            )
            nc.tensor.matmul(
                ps[:, 512:1024],
                k_sb[pair][DH:2 * DH, jc * P:(jc + 1) * P],
                q_sb[pair][DH:2 * DH, icq * 512:(icq + 1) * 512],
                start=True, stop=True, tile_position=(64, 0),
            )
            # expt double-buffered by stage parity: stage s+1's exps must
            # not overwrite the tiles stage s's AV matmuls still read.
            et = epool.tile([P, 1024], bf16, tag=f"e{jc}_{s % 2}",
                            name=f"e{jc}")
            if jc in DVE_JC:
                nc.vector.tensor_scalar(
                    et[:].bitcast(i16), ps[:], SCH_B, None, ALU.add)
            else:
                nc.scalar.activation(et[:], ps[:], AF.Exp, scale=1.0 / SCH_A)
            return et

        def emit_av(pair, icq, hh, expt, jlo, jhi, av):
            h = 2 * pair + hh
            for jc in range(jlo, jhi):
                nc.tensor.matmul(
                    av[:],
                    vt_sb[jc][:, h * (DH + 1):(h + 1) * (DH + 1)],
                    expt[jc][:, hh * 512:(hh + 1) * 512],
                    start=(jc == 0), stop=(jc == 7),
                )

        # Normalize chain split into three independently-emittable steps so
        # the DVE->Pool->DVE chain can be spaced a full pipeline stage after
        # its av producer: every step's deps are long-finished when its
        # engine dequeues it (strict-FIFO queues stall on head-of-line
        # dependencies otherwise).
        def emit_recip(av):
            rz = spool.tile([1, 512], f32, tag="rz", name="rz")
            nc.vector.reciprocal(rz[:], av[DH:DH + 1, :])
            return rz

        def emit_bcast(rz):
            bc = spool.tile([DH, 512], f32, tag="bc", name="bc")
            nc.gpsimd.partition_broadcast(bc[:], rz[:])
            return bc

        def emit_mult(pair, icq, hh, av, bc):
            dst = otn_sb[pair][hh * DH:(hh + 1) * DH,
                               icq * 512:(icq + 1) * 512]
            nc.vector.tensor_tensor(dst, av[0:DH, :], bc[:], ALU.mult)

        # 3-deep software pipeline over stages s = (pair, icq):
        #   stage s emission: sims+exps(s) | AVs(s-1) | normalize(s-LAG-1)
        # With PSUM3 the normalize lag drops to the same slot as the AVs
        # (av tiles live one slot, 2 psum bufs suffice, and the freed 2
        # banks give the sims a third psum buffer).
        LAG = 1 if PSUM3 else 2
        stages = [(pair, icq) for pair in range(4) for icq in range(4)]
        N = len(stages)
        av_of = {}     # s -> (avA, avB)
        rz_of = {}     # s -> (rzA, rzB)
        bc_of = {}     # s -> (bcA, bcB)
        expt_of = {}   # s -> {jc: tile}

        def emit_stage(s):
            """Emit one pipeline slot: interleave blocks so no engine queue
            head-of-line blocks on a same-slot dependency."""
            cur = stages[s] if s < N else None
            prv = stages[s - 1] if 1 <= s <= N else None
            o = s - LAG  # stage being normalized this slot
            old = stages[o] if 0 <= o < N else None
            if cur is not None:
                expt_of[s] = {}
            if prv is not None:
                avA = psAv.tile([DH + 1, 512], f32, tag="av", name="avA")
                avB = psAv.tile([DH + 1, 512], f32, tag="av", name="avB")
                av_of[s - 1] = (avA, avB)

            def sim2(j0):
                if cur is not None:
                    expt_of[s][j0] = emit_sim_exp(s, *cur, j0)
                    expt_of[s][j0 + 1] = emit_sim_exp(s, *cur, j0 + 1)

            sim2(0)
            if s == 0:
                emit_vt2(0)
            if prv is not None:
                emit_av(*prv, 0, expt_of[s - 1], 0, 4, av_of[s - 1][0])
            if old is not None and LAG == 2:
                rz_of[o] = (emit_recip(av_of[o][0]),
                            emit_recip(av_of[o][1]))
            sim2(2)
            if s == 0:
                emit_vt2(2)
            if prv is not None:
                emit_av(*prv, 0, expt_of[s - 1], 4, 8, av_of[s - 1][0])
            if old is not None:
                if LAG == 2:
                    bc_of[o] = (emit_bcast(rz_of[o][0]),
                                emit_bcast(rz_of[o][1]))
                else:
                    rz_of[o] = (emit_recip(av_of[o][0]), None)
            sim2(4)
            if s == 0:
                emit_vt2(4)
            if STAGGER and 1 <= s <= 9:
                fn, args = stagger[s - 1]
                fn(*args)
            if prv is not None:
                emit_av(*prv, 1, expt_of[s - 1], 0, 4, av_of[s - 1][1])
            if old is not None and LAG == 1:
                bc_of[o] = (emit_bcast(rz_of[o][0]), None)
            sim2(6)
            if s == 0:
                emit_vt2(6)
            if prv is not None:
                emit_av(*prv, 1, expt_of[s - 1], 4, 8, av_of[s - 1][1])
            if old is not None and LAG == 1:
                rz_of[o] = (rz_of[o][0], emit_recip(av_of[o][1]))
                bc_of[o] = (bc_of[o][0], emit_bcast(rz_of[o][1]))
            if old is not None:
                emit_mult(*old, 0, av_of[o][0], bc_of[o][0])
                emit_mult(*old, 1, av_of[o][1], bc_of[o][1])
                del av_of[o], expt_of[o]
            # output projection rides the pipeline tail: y chunk icq only
            # needs every pair's otn[:, icq], whose last mult is emitted at
            # stage 12+LAG+icq, so chunks stream out during the drain
            # stages instead of serializing after them.
            if s - 12 - LAG >= 0 and (icy := s - 12 - LAG) <= 3:
                emit_y(icy)
            # last slot of the LAG=2 schedule: fold the final normalize +
            # y chunk in instead of paying a whole extra drain slot.
            if LAG == 2 and s == N:
                rzl = (emit_recip(av_of[N - 1][0]),
                       emit_recip(av_of[N - 1][1]))
                bcl = (emit_bcast(rzl[0]), emit_bcast(rzl[1]))
                emit_mult(*stages[N - 1], 0, av_of[N - 1][0], bcl[0])
                emit_mult(*stages[N - 1], 1, av_of[N - 1][1], bcl[1])
                emit_y(3)

        def emit_y(icq):
            for mc in range(2):
                ps = psSim.tile([P, 1024], f32, tag="sim", name="yp")
                for pair in range(4):
                    nc.tensor.matmul(
                        ps[:, 0:512],
                        wo_sb[pair][:, mc * P:(mc + 1) * P],
                        otn_sb[pair][:, icq * 512:(icq + 1) * 512],
                        start=(pair == 0), stop=(pair == 3),
                    )
                nc.vector.tensor_scalar(
                    y_sb[mc][:, icq * 512:(icq + 1) * 512],
                    ps[:, 0:512], bo_sb[mc][:], None, ALU.add)
                nc.sync.dma_start(
                    y_d[mc * P:(mc + 1) * P, icq * 512:(icq + 1) * 512],
                    y_sb[mc][:, icq * 512:(icq + 1) * 512])

        y_sb = [wpool.tile([P, IC], f32, tag=f"y{mc}", name=f"y{mc}")
                for mc in range(2)]

        emit_proj_k(0)
        emit_proj_q(0, 0)
        emit_proj_q(0, 1)
        if not STAGGER:
            # all remaining projections in the head, copies on the DVE:
            # it has slack here, and every op billed to the ACT engine
            # (the exp bottleneck) costs slope time across For_i iterations
            for fn, args in stagger:
                fn(*args)
        for s in range(N + 1):
            emit_stage(s)

    with tile.TileContext(nc) as tc:
        with (
            tc.tile_pool(name="wpool", bufs=1) as wpool,
            tc.tile_pool(name="qpool", bufs=1) as qpool,
            tc.tile_pool(name="kpool", bufs=1) as kpool,
            tc.tile_pool(name="vpool", bufs=1) as vpool,
            tc.tile_pool(name="epool", bufs=1) as epool,
            tc.tile_pool(name="opool", bufs=1) as opool,
            tc.tile_pool(name="spool", bufs=4) as spool,
            tc.tile_pool(name="psSim", bufs=(3 if PSUM3 else 2),
                         space="PSUM") as psSim,
            tc.tile_pool(name="psAv", bufs=(2 if PSUM3 else 4),
                         space="PSUM") as psAv,
        ):
            pools = (wpool, qpool, kpool, vpool, epool, opool, spool,
                     psSim, psAv)
            if reps == 1:
                body(nc, tc, pools)
            else:
                with tc.For_i(0, reps, 1):
                    body(nc, tc, pools)

    nc.compile()
    nc.m = get_hw_module(nc.m)
    return nc


def _shard_inputs(x, a, Wq, Wkv, Wout, bout):
    xf = np.ascontiguousarray(x.reshape(B, CQ, HW), dtype=np.float32)
    af = np.ascontiguousarray(a.reshape(B, CKV, NJ), dtype=np.float32)
    # attention scale and the Schraudolph exp scale both fold into Wq
    wq = np.ascontiguousarray((Wq * (DH ** -0.5 * SCH_A)).T, dtype=np.float32)
    wk = np.ascontiguousarray(Wkv[:HID].T, dtype=np.float32)
    wv = np.ascontiguousarray(Wkv[HID:].T, dtype=np.float32)
    wo = np.ascontiguousarray(Wout.T, dtype=np.float32)
    bo = np.ascontiguousarray(bout.reshape(CQ, 1), dtype=np.float32)
    in_maps = []
    for c in range(8):
        b, half = c // 2, c % 2
        in_maps.append({
            "x": np.ascontiguousarray(xf[b][:, half * IC:(half + 1) * IC]),
            "a": af[b],
            "wq": wq, "wk": wk, "wv": wv, "wo": wo, "bo": bo,
            "ones": np.ones((P, HEADS), dtype=np.float32),
        })
    return in_maps


def _get_runner():
    global _RUNNER
    if _RUNNER is None:
        _RUNNER = _build_nc()
    return _RUNNER


_JIT = None


def _make_jit(nc):
    """Build a sharded PJRT callable for a compiled nc."""
    import jax
    import concourse.mybir as mybir
    from jax.sharding import Mesh, PartitionSpec
    from jax.experimental.shard_map import shard_map
    from concourse.bass2jax import (
        _bass_exec_p, install_neuronx_cc_hook, partition_id_tensor)

    install_neuronx_cc_hook()
    partition_name = (
        nc.partition_id_tensor.name if nc.partition_id_tensor else None)
    in_names, out_names, out_avals, zero_outs = [], [], [], []
    for alloc in nc.m.functions[0].allocations:
        if not isinstance(alloc, mybir.MemoryLocationSet):
            continue
        name = alloc.memorylocations[0].name
        if alloc.kind == "ExternalInput":
            if name != partition_name:
                in_names.append(name)
        elif alloc.kind == "ExternalOutput":
            shape = tuple(alloc.tensor_shape)
            dtype = mybir.dt.np(alloc.dtype)
            out_names.append(name)
            out_avals.append(jax.core.ShapedArray(shape, dtype))
            zero_outs.append((shape, dtype))
    n_params = len(in_names)
    all_in_names = list(in_names) + list(out_names)
    if partition_name is not None:
        all_in_names.append(partition_name)

    def _body(*args):
        operands = list(args)
        if partition_name is not None:
            operands.append(partition_id_tensor())
        outs = _bass_exec_p.bind(
            *operands,
            out_avals=tuple(out_avals),
            in_names=tuple(all_in_names),
            out_names=tuple(out_names),
            lowering_input_output_aliases=(),
            sim_require_finite=True,
            sim_require_nnan=True,
            nc=nc,
        )
        return tuple(outs)

    devices = jax.devices()[:8]
    mesh = Mesh(np.asarray(devices), ("core",))
    in_specs = (PartitionSpec("core"),) * (n_params + len(out_names))
    out_specs = (PartitionSpec("core"),) * len(out_names)
    sharded = jax.jit(
        shard_map(_body, mesh=mesh, in_specs=in_specs, out_specs=out_specs,
                  check_rep=False),
        keep_unused=True)
    return (sharded, in_names, out_names, out_avals, zero_outs)


def _get_jit():
    global _JIT
    if _JIT is None:
        _JIT = _make_jit(_get_runner())
    return _JIT


_DEV_CACHE = {"fp": None, "dev_in": None, "dev_zeros": None}


def _stage_inputs(concat_in, zero_outs):
    """device_put inputs once; reuse when the same bytes are passed again."""
    import jax
    import zlib
    fp = tuple(zlib.adler32(a.tobytes()) for a in concat_in)
    if _DEV_CACHE["fp"] != fp or _DEV_CACHE["dev_in"] is None:
        _DEV_CACHE["dev_in"] = [jax.device_put(a) for a in concat_in]
        _DEV_CACHE["fp"] = fp
    if _DEV_CACHE["dev_zeros"] is None:
        _DEV_CACHE["dev_zeros"] = [
            jax.device_put(np.zeros((8 * s[0], *s[1:]), d))
            for (s, d) in zero_outs
        ]
    return _DEV_CACHE["dev_in"], _DEV_CACHE["dev_zeros"]


def run_sharded(in_maps):
    """Run the SPMD kernel; returns list of per-core output dicts."""
    sharded, in_names, out_names, out_avals, zero_outs = _get_jit()
    concat_in = [
        np.ascontiguousarray(
            np.concatenate([np.asarray(m[name]) for m in in_maps], axis=0))
        for name in in_names
    ]
    dev_in, dev_zeros = _stage_inputs(concat_in, zero_outs)
    out_arrs = sharded(*dev_in, *dev_zeros)
    return [
        {name: np.asarray(out_arrs[i]).reshape(8, *out_avals[i].shape)[c]
         for i, name in enumerate(out_names)}
        for c in range(8)
    ]


def run_staged():
    """Re-run with already-staged device inputs (timing helper)."""
    sharded, in_names, out_names, out_avals, zero_outs = _get_jit()
    out = sharded(*_DEV_CACHE["dev_in"], *_DEV_CACHE["dev_zeros"])
    for o in out:
        o.block_until_ready()
    return out


def kernel(x, a, Wq, Wkv, Wout, bout):
    in_maps = _shard_inputs(
        np.asarray(x), np.asarray(a), np.asarray(Wq), np.asarray(Wkv),
        np.asarray(Wout), np.asarray(bout))
    results = run_sharded(in_maps)
    y = np.empty((B, CQ, HW), dtype=np.float32)
    for c in range(8):
        b, half = c // 2, c % 2
        y[b][:, half * IC:(half + 1) * IC] = results[c]["y"]
    return y.reshape(B, CQ, 64, 64)
